# revision 11
# baseline (speedup 1.0000x reference)
"""DIEN (GRU + AUGRU scan) Trainium2 Bass kernel.

Strategy
--------
Data-parallel over batch: B=256 is split 8 ways (32 per core). All weights are
replicated. The sequential scan over T=200 runs locally per core.

Algebraic fusion (host-side, exact):
  The per-step attention is over a length-1 sequence, so softmax==1 and
  attn == v_proj(g). Folding v_proj into the AUGRU input weights:
    aug_in @ augru_Wih.T == g @ (A1 + A2 @ v_W).T + (augru_bih + A2 @ v_b)
  with A1 = augru_Wih[:, :H], A2 = augru_Wih[:, H:]. This removes the v-proj
  matmul and halves the AUGRU input GEMM.

Matmuls run in bf16 (PSUM accumulation stays fp32): the TRN2 PE pumps bf16 at
4x the fp32 rate. seq_emb is pre-transposed and cast to bf16 on the host so
each step's stationary operand DMAs straight into the K-major layout.

Per-step compute per core: 4 GEMM groups of [32,512] @ [512,1536]
(x-projection, GRU-hidden, fused AUGRU-input, AUGRU-hidden). Each group is
mapped PE-efficiently with the batch (32) as the stationary free dim using
4x column tiling (tile_position=(0,32c)): 4 concurrent matmuls per K-tile,
each pumping 384 weight columns.

The emission order software-pipelines the recurrence: matmuls that do not
depend on the freshest state (x-projection of step t+1, AUGRU-hidden of step
t) are emitted before the gT(t)-dependent ones so the PE pre-pumps them while
the GRU cell's elementwise chain runs.

Layouts (per core, batch b in 0..31, hidden h = 128*c + 32*m + jr):
  row layout  : tile[32*c + b, 32*m + jr]  (states, gates, psum outputs)
  stationary  : tileT[32*c + jr, 32*m + b] -- obtained from row layout by a
                single DVE 32x32 block transpose (with f32->bf16 cast); K-tile
                m of the GEMM contracts hidden dims {128c + 32m + jr}, and the
                weight matrices are pre-arranged (host-side numpy) to match.
"""

import os
import sys

import numpy as np
import ml_dtypes

for _p in ("/opt/trn_rl_repo", "/root/.axon_site/_ro/trn_rl_repo"):
    if os.path.isdir(_p) and _p not in sys.path:
        sys.path.append(_p)

BF16 = ml_dtypes.bfloat16

B, T, H = 256, 200, 512
N_CORES = 8
BL = B // N_CORES  # 32

_CACHE = {}


# ---------------------------------------------------------------------------
# Host-side weight preparation (pure numpy, exact rearrangements)
# ---------------------------------------------------------------------------

def _arrange_w(W):
    """[3H, H] (out, in) -> [128, 4, 1536] K-tile-arranged weight blocks (bf16).

    Block m, partition p = 32*c_in + jr holds input dim h_in = 128*c_in + 32*m + jr.
    Free index f = c_out*384 + gate*128 + j  maps output col gate*512 + c_out*128 + j.
    """
    A = W.T.reshape(4, 4, 32, 3 * H)                # [c_in, m, jr, out]
    A = A.transpose(1, 0, 2, 3).reshape(4, 128, 3 * H)
    A = A.reshape(4, 128, 3, 4, 128).transpose(0, 1, 3, 2, 4).reshape(4, 128, 3 * H)
    A = A.transpose(1, 0, 2)                        # [p, m, out] for contiguous DMA
    return np.ascontiguousarray(A).astype(BF16)


def _arrange_seq(seq_core):
    """[BL, T, H] f32 -> [T, 128, 128] bf16 in stationary (K-major) layout.

    Out[t, 32*c + jr, 32*m + b] = seq[b, t, 128*c + 32*m + jr].
    """
    A = seq_core.reshape(BL, T, 4, 4, 32)           # [b, t, c, m, jr]
    A = A.transpose(1, 2, 4, 3, 0)                  # [t, c, jr, m, b]
    return np.ascontiguousarray(A.reshape(T, 128, 128)).astype(BF16)


def _bias_row(b_rz, b_ihn, b_hhn):
    """Bias vectors -> [1, 2048] bf16 row for the bank-starting ones-matmul.

    Strip c consumes cols [512c : 512c+512] = [rz(256) | n_i(128) | n_h(128)]
    matching the cell's PSUM bank layout.
    """
    rz = b_rz[:1024].reshape(2, 4, 128).transpose(1, 0, 2).reshape(4, 256)
    ihn = b_ihn[1024:].reshape(4, 128)
    hhn = b_hhn[1024:].reshape(4, 128)
    row = np.concatenate(
        [np.concatenate([rz[c], ihn[c], hhn[c]]) for c in range(4)]
    )
    return np.ascontiguousarray(row[None, :]).astype(BF16)


# ---------------------------------------------------------------------------
# Bass program
# ---------------------------------------------------------------------------

def _build_program(n_steps=T, mm_f32=False):
    import concourse.bacc as bacc
    import concourse.tile as tile
    from concourse import mybir
    from contextlib import ExitStack

    F32 = mybir.dt.float32
    BF = mybir.dt.float32 if mm_f32 else mybir.dt.bfloat16
    Sigmoid = mybir.ActivationFunctionType.Sigmoid
    Tanh = mybir.ActivationFunctionType.Tanh

    nc = bacc.Bacc("TRN2", target_bir_lowering=False, debug=False)

    seq = nc.declare_dram_parameter("seq", [n_steps, 128, 128], BF, isOutput=False)
    w_dram = {
        name: nc.declare_dram_parameter(name, [128, 4, 3 * H], BF, isOutput=False)
        for name in ("wgi", "wgh", "wai", "wah")
    }
    b_dram = {}
    for name, cols in (
        ("brz_g", 256), ("bihn_g", 128), ("bhhn_g", 128),
        ("brz_a", 256), ("bihn_a", 128), ("bhhn_a", 128),
    ):
        b_dram[name] = nc.declare_dram_parameter(name, [128, cols], F32, isOutput=False)
    out = nc.declare_dram_parameter("out", [BL, H], F32, isOutput=True)

    with tile.TileContext(nc) as tc, ExitStack() as ctx:
        wpool = ctx.enter_context(tc.tile_pool(name="weights", bufs=1))
        xt_pool = ctx.enter_context(tc.tile_pool(name="xt", bufs=6))
        st_pool = ctx.enter_context(tc.tile_pool(name="states", bufs=3))
        tmp_pool = ctx.enter_context(tc.tile_pool(name="tmps", bufs=3))
        psum_pool = ctx.enter_context(tc.tile_pool(name="psum", bufs=2, space="PSUM"))

        # --- constants: weights + biases ---
        wsb = {}
        for name, drm in w_dram.items():
            t = wpool.tile([128, 4 * 3 * H], BF, tag=name)
            nc.sync.dma_start(out=t, in_=drm[:].rearrange("p m f -> p (m f)"))
            wsb[name] = t
        bsb = {}
        for name, drm in b_dram.items():
            t = wpool.tile([128, drm.shape[1]], F32, tag=name)
            nc.sync.dma_start(out=t, in_=drm[:])
            bsb[name] = t

        # --- initial states (zero) ---
        # Rows are bf16: the GEMMs consume bf16 state anyway, so keeping the
        # row in bf16 (rather than f32 + a cast before the transpose) costs
        # only second-order precision while keeping the transpose same-dtype.
        g_row = st_pool.tile([128, 128], BF, tag="g_row")
        gT = st_pool.tile([128, 128], BF, tag="g_rowT")
        a_row = st_pool.tile([128, 128], BF, tag="a_row")
        aT = st_pool.tile([128, 128], BF, tag="a_rowT")
        for t_ in (g_row, gT, a_row, aT):
            nc.vector.memset(t_, 0.0)

        # Each cell-step's gates live in ONE full-bank PSUM tile [128, 512]
        # (2048 B/partition, so partition rows align 1:1 with PSUM zero-region
        # rows): cols 0:256 = rz (input+hidden accumulated), 256:384 = n input
        # side, 384:512 = n hidden side. The first matmul touching the bank
        # carries start=True, which marks the whole bank pending-zero; later
        # first-touch writes of other regions zero-write via the pending flags
        # (start=False), and re-writes accumulate.

        def mm_group(psum, statT, w, split_first=False):
            """psum[32c+b, 0:384] += statT-K-tiles.T @ w chunks (full r|z|n).

            split_first: an earlier hidden-side group already started this
            bank and accumulated into [0:256]; split the k0 write so each
            matmul touches all-pending ([256:384]) or none-pending ([0:256])
            bytes.
            """
            for k in range(4):
                lhsT = statT[:, 32 * k:32 * k + 32]
                for c in range(4):
                    base = k * 1536 + 384 * c
                    if split_first and k == 0:
                        nc.tensor.matmul(
                            out=psum[32 * c:32 * c + 32, 0:256],
                            lhsT=lhsT,
                            rhs=w[:, base:base + 256],
                            start=False, stop=False, skip_group_check=True,
                            tile_position=(0, 32 * c),
                        )
                        nc.tensor.matmul(
                            out=psum[32 * c:32 * c + 32, 256:384],
                            lhsT=lhsT,
                            rhs=w[:, base + 256:base + 384],
                            start=False, stop=False, skip_group_check=True,
                            tile_position=(0, 32 * c),
                        )
                    else:
                        nc.tensor.matmul(
                            out=psum[32 * c:32 * c + 32, 0:384],
                            lhsT=lhsT,
                            rhs=w[:, base:base + 384],
                            start=(not split_first and k == 0),
                            stop=(split_first and k == 3),
                            skip_group_check=True,
                            tile_position=(0, 32 * c),
                        )

        def mm_group_hh(psum, statT, w, starts=False):
            """Hidden-side group: r,z part accumulates onto [0:256] of the
            shared bank tile; n part goes to [384:512] so r can gate it before
            the tanh.

            starts: this group executes before the input-side group on the PE,
            so it owns the bank's start flag (k0 rz start=True); its k0 n write
            first-touch zero-writes via the pending flags.

            Emission order matters: the PE is strict FIFO and a matmul on col
            tile c blocks behind an unfinished matmul on the same tile, so we
            sweep c within each pump type to keep the 4 col tiles concurrent.
            """
            for k in range(4):
                lhsT = statT[:, 32 * k:32 * k + 32]
                for c in range(4):
                    base = k * 1536 + 384 * c
                    nc.tensor.matmul(
                        out=psum[32 * c:32 * c + 32, 0:256],
                        lhsT=lhsT,
                        rhs=w[:, base:base + 256],
                        start=(starts and k == 0),
                        stop=(not starts and k == 3),
                        skip_group_check=True,
                        tile_position=(0, 32 * c),
                    )
                for c in range(4):
                    base = k * 1536 + 384 * c
                    nc.tensor.matmul(
                        out=psum[32 * c:32 * c + 32, 384:512],
                        lhsT=lhsT,
                        rhs=w[:, base + 256:base + 384],
                        start=False, stop=(k == 3), skip_group_check=True,
                        tile_position=(0, 32 * c),
                    )

        def cell(pg, row_prev, brz, bihn, bhhn, row_tag):
            # pg[:, 0:256] holds gi_rz + gh_rz (PE-accumulated); pg[:, 256:384]
            # the input-side n; pg[:, 384:512] the hidden-side n.
            s2 = tmp_pool.tile([128, 256], F32, tag=row_tag + "s2")
            nc.vector.tensor_add(s2, pg[:, 0:256], brz)
            rz = tmp_pool.tile([128, 256], F32, tag=row_tag + "rz")
            nc.scalar.activation(rz, s2, Sigmoid)
            u = tmp_pool.tile([128, 128], F32, tag=row_tag + "u")
            nc.vector.tensor_add(u, pg[:, 384:512], bhhn)
            v = tmp_pool.tile([128, 128], F32, tag=row_tag + "v")
            nc.vector.tensor_mul(v, rz[:, 0:128], u)
            w0 = tmp_pool.tile([128, 128], F32, tag=row_tag + "w0")
            nc.vector.tensor_add(w0, pg[:, 256:384], bihn)
            t3 = tmp_pool.tile([128, 128], F32, tag=row_tag + "t3")
            nc.vector.tensor_add(t3, w0, v)
            n = tmp_pool.tile([128, 128], F32, tag=row_tag + "n")
            nc.scalar.activation(n, t3, Tanh)
            ew = nc.gpsimd
            d = tmp_pool.tile([128, 128], F32, tag=row_tag + "d")
            ew.tensor_sub(d, row_prev, n)
            e = tmp_pool.tile([128, 128], F32, tag=row_tag + "e")
            ew.tensor_mul(e, rz[:, 128:256], d)
            row_new = st_pool.tile([128, 128], BF, tag=row_tag)
            ew.tensor_add(row_new, n, e)
            rowT = st_pool.tile([128, 128], BF, tag=row_tag + "T")
            nc.vector.transpose(rowT, row_new)
            return row_new, rowT

        def load_x(t_):
            xT = xt_pool.tile([128, 128], BF, tag="xT")
            nc.sync.dma_start(out=xT, in_=seq[t_, :, :])
            return xT

        # --- software-pipelined scan ---
        # Prologue: step 0's x-projection + GRU-hidden (on zero state).
        xT = load_x(0)
        pg = psum_pool.tile([128, 512], F32, tag="gi")
        mm_group(pg, xT, wsb["wgi"])
        mm_group_hh(pg, gT, wsb["wgh"])
        g_row, gT = cell(
            pg, g_row, bsb["brz_g"], bsb["bihn_g"], bsb["bhhn_g"], "g_row"
        )

        for t_ in range(n_steps):
            # AUGRU step t: hidden-side first (depends only on aT(t-1), so the
            # PE pre-pumps it while the GRU cell's elementwise chain runs).
            pa = psum_pool.tile([128, 512], F32, tag="ai")
            mm_group_hh(pa, aT, wsb["wah"], starts=True)

            if t_ + 1 < n_steps:
                # GRU step t+1 x-projection: ready early, pre-pumps too.
                xT = load_x(t_ + 1)
                pg = psum_pool.tile([128, 512], F32, tag="gi")
                mm_group(pg, xT, wsb["wgi"])

            # gT(t)-dependent matmuls.
            mm_group(pa, gT, wsb["wai"], split_first=True)
            if t_ + 1 < n_steps:
                mm_group_hh(pg, gT, wsb["wgh"])
                # GRU cell t+1 ahead of AUGRU cell t: its output feeds the
                # next iteration's critical path.
                g_row, gT = cell(
                    pg, g_row,
                    bsb["brz_g"], bsb["bihn_g"], bsb["bhhn_g"], "g_row"
                )

            a_row, aT = cell(
                pa, a_row, bsb["brz_a"], bsb["bihn_a"], bsb["bhhn_a"], "a_row"
            )

        import concourse.bass as bass_mod

        a_f32 = tmp_pool.tile([128, 128], F32, tag="a_f32")
        nc.scalar.activation(a_f32, a_row, mybir.ActivationFunctionType.Copy)
        out_ap = bass_mod.AP(
            tensor=out[:].tensor,
            offset=0,
            ap=[[128, 4], [H, BL], [1, 128]],
        )
        nc.sync.dma_start(out=out_ap, in_=a_f32)

    nc.compile()
    return nc


def _get_program(n_steps=T):
    key = ("prog", n_steps)
    if key not in _CACHE:
        _CACHE[key] = _build_program(n_steps)
    return _CACHE[key]


# ---------------------------------------------------------------------------
# Entry point
# ---------------------------------------------------------------------------

def _make_in_maps(inputs):
    seq_emb = np.asarray(inputs["seq_emb"], np.float32)
    augru_Wih = np.asarray(inputs["augru_Wih"])
    A1 = augru_Wih[:, :H]
    A2 = augru_Wih[:, H:]
    w_fused = (A1 + A2 @ np.asarray(inputs["v_W"])).astype(np.float32)
    b_ai = (np.asarray(inputs["augru_bih"]) + A2 @ np.asarray(inputs["v_b"])).astype(np.float32)
    b_ah = np.asarray(inputs["augru_bhh"], np.float32)
    gru_bih = np.asarray(inputs["gru_bih"])
    gru_bhh = np.asarray(inputs["gru_bhh"])

    consts = {
        "wgi": _arrange_w(np.asarray(inputs["gru_Wih"])),
        "wgh": _arrange_w(np.asarray(inputs["gru_Whh"])),
        "wai": _arrange_w(w_fused),
        "wah": _arrange_w(np.asarray(inputs["augru_Whh"])),
        "brz_g": _bias_rz(gru_bih + gru_bhh),
        "bihn_g": _bias_n(gru_bih),
        "bhhn_g": _bias_n(gru_bhh),
        "brz_a": _bias_rz(b_ai + b_ah),
        "bihn_a": _bias_n(b_ai),
        "bhhn_a": _bias_n(b_ah),
    }
    return [
        {"seq": _arrange_seq(seq_emb[c * BL:(c + 1) * BL]), **consts}
        for c in range(N_CORES)
    ]


def _prep_and_run(trace=False, **inputs):
    from concourse.bass_utils import run_bass_kernel_spmd

    in_maps = _make_in_maps(inputs)
    nc = _get_program()
    res = run_bass_kernel_spmd(nc, in_maps, list(range(N_CORES)), trace=trace)
    out = np.concatenate([res.results[c]["out"] for c in range(N_CORES)], axis=0)
    return out.astype(np.float32), res


def kernel(**inputs):
    out, _ = _prep_and_run(**inputs)
    return out


def kernel_traced(**inputs):
    """Like kernel() but profiles the run; returns (output, BassKernelResults)."""
    return _prep_and_run(**inputs, trace=True)


if __name__ == "__main__":
    rng = np.random.default_rng(0)
    ins = {
        "seq_emb": rng.standard_normal((B, T, H), dtype=np.float32),
        "target_emb": rng.standard_normal((B, H), dtype=np.float32),
        "gru_Wih": rng.standard_normal((3 * H, H), dtype=np.float32) * 0.04,
        "gru_Whh": rng.standard_normal((3 * H, H), dtype=np.float32) * 0.04,
        "gru_bih": rng.standard_normal(3 * H).astype(np.float32) * 0.04,
        "gru_bhh": rng.standard_normal(3 * H).astype(np.float32) * 0.04,
        "q_W": rng.standard_normal((H, H), dtype=np.float32) * 0.04,
        "q_b": rng.standard_normal(H).astype(np.float32) * 0.04,
        "k_W": rng.standard_normal((H, H), dtype=np.float32) * 0.04,
        "k_b": rng.standard_normal(H).astype(np.float32) * 0.04,
        "v_W": rng.standard_normal((H, H), dtype=np.float32) * 0.04,
        "v_b": rng.standard_normal(H).astype(np.float32) * 0.04,
        "augru_Wih": rng.standard_normal((3 * H, 2 * H), dtype=np.float32) * 0.04,
        "augru_Whh": rng.standard_normal((3 * H, H), dtype=np.float32) * 0.04,
        "augru_bih": rng.standard_normal(3 * H).astype(np.float32) * 0.04,
        "augru_bhh": rng.standard_normal(3 * H).astype(np.float32) * 0.04,
    }
    o = kernel(**ins)
    print("kernel output", o.shape, o.dtype, float(np.abs(o).max()))


# revision 20
# speedup vs baseline: 1.3566x; 1.3566x over previous
"""DIEN (GRU + AUGRU scan) Trainium2 Bass kernel.

Strategy
--------
Data-parallel over batch: B=256 is split 8 ways (32 per core). All weights are
replicated. The sequential scan over T=200 runs locally per core.

Algebraic fusion (host-side, exact):
  The per-step attention is over a length-1 sequence, so softmax==1 and
  attn == v_proj(g). Folding v_proj into the AUGRU input weights:
    aug_in @ augru_Wih.T == g @ (A1 + A2 @ v_W).T + (augru_bih + A2 @ v_b)
  with A1 = augru_Wih[:, :H], A2 = augru_Wih[:, H:]. This removes the v-proj
  matmul and halves the AUGRU input GEMM.

Matmuls run in bf16 (PSUM accumulation stays fp32): the TRN2 PE pumps bf16 at
4x the fp32 rate. seq_emb is pre-transposed and cast to bf16 on the host so
each step's stationary operand DMAs straight into the K-major layout.

Per-step compute per core: 4 GEMM groups of [32,512] @ [512,1536]
(x-projection, GRU-hidden, fused AUGRU-input, AUGRU-hidden). Each group is
mapped PE-efficiently with the batch (32) as the stationary free dim using
4x column tiling (tile_position=(0,32c)): 4 concurrent matmuls per K-tile,
each pumping 384 weight columns.

The emission order software-pipelines the recurrence: matmuls that do not
depend on the freshest state (x-projection of step t+1, AUGRU-hidden of step
t) are emitted before the gT(t)-dependent ones so the PE pre-pumps them while
the GRU cell's elementwise chain runs.

Layouts (per core, batch b in 0..31, hidden h = 128*c + 32*m + jr):
  row layout  : tile[32*c + b, 32*m + jr]  (states, gates, psum outputs)
  stationary  : tileT[32*c + jr, 32*m + b] -- obtained from row layout by a
                single DVE 32x32 block transpose (with f32->bf16 cast); K-tile
                m of the GEMM contracts hidden dims {128c + 32m + jr}, and the
                weight matrices are pre-arranged (host-side numpy) to match.
"""

import os
import sys

import numpy as np
import ml_dtypes

for _p in ("/opt/trn_rl_repo", "/root/.axon_site/_ro/trn_rl_repo"):
    if os.path.isdir(_p) and _p not in sys.path:
        sys.path.append(_p)

BF16 = ml_dtypes.bfloat16

B, T, H = 256, 200, 512
N_CORES = 8
BL = B // N_CORES  # 32

_CACHE = {}


# ---------------------------------------------------------------------------
# Host-side weight preparation (pure numpy, exact rearrangements)
# ---------------------------------------------------------------------------

def _arrange_w(W):
    """[3H, H] (out, in) -> [128, 4, 1536] K-tile-arranged weight blocks (bf16).

    Block m, partition p = 32*c_in + jr holds input dim h_in = 128*c_in + 32*m + jr.
    Free index f = c_out*384 + gate*128 + j  maps output col gate*512 + c_out*128 + j.
    """
    A = W.T.reshape(4, 4, 32, 3 * H)                # [c_in, m, jr, out]
    A = A.transpose(1, 0, 2, 3).reshape(4, 128, 3 * H)
    A = A.reshape(4, 128, 3, 4, 128).transpose(0, 1, 3, 2, 4).reshape(4, 128, 3 * H)
    A = A.transpose(1, 0, 2)                        # [p, m, out] for contiguous DMA
    return np.ascontiguousarray(A).astype(BF16)


def _arrange_seq(seq_core):
    """[BL, T, H] f32 -> [T, 128, 128] bf16 in stationary (K-major) layout.

    Out[t, 32*c + jr, 32*m + b] = seq[b, t, 128*c + 32*m + jr].
    """
    A = seq_core.reshape(BL, T, 4, 4, 32)           # [b, t, c, m, jr]
    A = A.transpose(1, 2, 4, 3, 0)                  # [t, c, jr, m, b]
    return np.ascontiguousarray(A.reshape(T, 128, 128)).astype(BF16)


def _bias_row(b_rz, b_ihn, b_hhn):
    """Bias vectors -> [1, 2048] bf16 row for the bank-starting ones-matmul.

    Strip c consumes cols [512c : 512c+512] = [rz(256) | n_i(128) | n_h(128)]
    matching the cell's PSUM bank layout.
    """
    rz = b_rz[:1024].reshape(2, 4, 128).transpose(1, 0, 2).reshape(4, 256)
    ihn = b_ihn[1024:].reshape(4, 128)
    hhn = b_hhn[1024:].reshape(4, 128)
    row = np.concatenate(
        [np.concatenate([rz[c], ihn[c], hhn[c]]) for c in range(4)]
    )
    return np.ascontiguousarray(row[None, :]).astype(BF16)


# ---------------------------------------------------------------------------
# Bass program
# ---------------------------------------------------------------------------

def _build_program(n_steps=T, mm_f32=False):
    import concourse.bacc as bacc
    import concourse.tile as tile
    from concourse import mybir
    from contextlib import ExitStack

    F32 = mybir.dt.float32
    BF = mybir.dt.float32 if mm_f32 else mybir.dt.bfloat16
    Sigmoid = mybir.ActivationFunctionType.Sigmoid
    Tanh = mybir.ActivationFunctionType.Tanh

    nc = bacc.Bacc("TRN2", target_bir_lowering=False, debug=False)

    # seq always holds T steps; programs with n_steps > T wrap around (used
    # only by test.py's slope timing, which needs longer programs with
    # identical input bytes).
    seq = nc.declare_dram_parameter("seq", [T, 128, 128], BF, isOutput=False)
    w_dram = {
        name: nc.declare_dram_parameter(name, [128, 4, 3 * H], BF, isOutput=False)
        for name in ("wgi", "wgh", "wai", "wah")
    }
    b_dram = {
        name: nc.declare_dram_parameter(name, [1, 2048], BF, isOutput=False)
        for name in ("bias_g", "bias_a")
    }
    out = nc.declare_dram_parameter("out", [BL, H], F32, isOutput=True)

    with tile.TileContext(nc) as tc, ExitStack() as ctx:
        wpool = ctx.enter_context(tc.tile_pool(name="weights", bufs=1))
        xt_pool = ctx.enter_context(tc.tile_pool(name="xt", bufs=6))
        st_pool = ctx.enter_context(tc.tile_pool(name="states", bufs=3))
        tmp_pool = ctx.enter_context(tc.tile_pool(name="tmps", bufs=3))
        psum_pool = ctx.enter_context(tc.tile_pool(name="psum", bufs=2, space="PSUM"))

        # --- constants: weights + biases ---
        wsb = {}
        for name, drm in w_dram.items():
            t = wpool.tile([128, 4 * 3 * H], BF, tag=name)
            nc.sync.dma_start(out=t, in_=drm[:].rearrange("p m f -> p (m f)"))
            wsb[name] = t
        bsb = {}
        for name, drm in b_dram.items():
            t = wpool.tile([1, 2048], BF, tag=name)
            nc.sync.dma_start(out=t, in_=drm[:])
            bsb[name] = t
        ones = wpool.tile([1, 32], BF, tag="ones")
        nc.vector.memset(ones, 1.0)

        # --- initial states (zero) ---
        # Rows are bf16: the GEMMs consume bf16 state anyway, so keeping the
        # row in bf16 (rather than f32 + a cast before the transpose) costs
        # only second-order precision while keeping the transpose same-dtype.
        g_row = st_pool.tile([128, 128], BF, tag="g_row")
        gT = st_pool.tile([128, 128], BF, tag="g_rowT")
        a_row = st_pool.tile([128, 128], BF, tag="a_row")
        aT = st_pool.tile([128, 128], BF, tag="a_rowT")
        for t_ in (g_row, gT, a_row, aT):
            nc.vector.memset(t_, 0.0)

        # Each cell-step's gates live in ONE full-bank PSUM tile [128, 512]
        # (2048 B/partition, so partition rows align 1:1 with PSUM zero-region
        # rows): cols 0:256 = rz (input+hidden accumulated), 256:384 = n input
        # side, 384:512 = n hidden side. A bank-starting ones-matmul writes the
        # biases into the whole bank with start=True (marking + consuming the
        # pending-zero flags in one go); every gate matmul then accumulates
        # with start=False. This folds all 6 bias adds into the PE.

        def mm_bias(psum, bias_sb):
            for c in range(4):
                nc.tensor.matmul(
                    out=psum[32 * c:32 * c + 32, 0:512],
                    lhsT=ones,
                    rhs=bias_sb[:, 512 * c:512 * c + 512],
                    start=True, stop=False, skip_group_check=True,
                    tile_position=(0, 32 * c),
                )

        def mm_group(psum, statT, w):
            """psum[32c+b, 0:384] += statT-K-tiles.T @ w chunks (full r|z|n)."""
            for k in range(4):
                lhsT = statT[:, 32 * k:32 * k + 32]
                for c in range(4):
                    base = k * 1536 + 384 * c
                    nc.tensor.matmul(
                        out=psum[32 * c:32 * c + 32, 0:384],
                        lhsT=lhsT,
                        rhs=w[:, base:base + 384],
                        start=False, stop=(k == 3), skip_group_check=True,
                        tile_position=(0, 32 * c),
                    )

        def mm_group_hh(psum, statT, w):
            """Hidden-side group: r,z part accumulates onto [0:256] of the
            shared bank tile; n part goes to [384:512] so r can gate it before
            the tanh.

            Emission order matters: the PE is strict FIFO and a matmul on col
            tile c blocks behind an unfinished matmul on the same tile, so we
            sweep c within each pump type to keep the 4 col tiles concurrent.
            """
            for k in range(4):
                lhsT = statT[:, 32 * k:32 * k + 32]
                for c in range(4):
                    base = k * 1536 + 384 * c
                    nc.tensor.matmul(
                        out=psum[32 * c:32 * c + 32, 0:256],
                        lhsT=lhsT,
                        rhs=w[:, base:base + 256],
                        start=False, stop=(k == 3), skip_group_check=True,
                        tile_position=(0, 32 * c),
                    )
                for c in range(4):
                    base = k * 1536 + 384 * c
                    nc.tensor.matmul(
                        out=psum[32 * c:32 * c + 32, 384:512],
                        lhsT=lhsT,
                        rhs=w[:, base + 256:base + 384],
                        start=False, stop=(k == 3), skip_group_check=True,
                        tile_position=(0, 32 * c),
                    )

        def cell(pg, row_prev, row_tag):
            # pg[:, 0:256] holds rz (input+hidden+bias, PE-accumulated);
            # pg[:, 256:384] the input-side n (+bias); pg[:, 384:512] the
            # hidden-side n (+bias). Chain: ACT -> Pool -> ACT -> DVE, with
            # the tail + transpose colocated on DVE (no semaphore between).
            rz = tmp_pool.tile([128, 256], BF, tag=row_tag + "rz")
            nc.scalar.activation(rz, pg[:, 0:256], Sigmoid)
            v = tmp_pool.tile([128, 128], F32, tag=row_tag + "v")
            nc.vector.tensor_mul(v, rz[:, 0:128], pg[:, 384:512])
            t3 = tmp_pool.tile([128, 128], F32, tag=row_tag + "t3")
            nc.vector.tensor_add(t3, v, pg[:, 256:384])
            n = tmp_pool.tile([128, 128], BF, tag=row_tag + "n")
            nc.scalar.activation(n, t3, Tanh)
            d = tmp_pool.tile([128, 128], BF, tag=row_tag + "d")
            nc.vector.tensor_sub(d, row_prev, n)
            e = tmp_pool.tile([128, 128], BF, tag=row_tag + "e")
            nc.vector.tensor_mul(e, rz[:, 128:256], d)
            row_new = st_pool.tile([128, 128], BF, tag=row_tag)
            nc.vector.tensor_add(row_new, n, e)
            rowT = st_pool.tile([128, 128], BF, tag=row_tag + "T")
            nc.vector.transpose(rowT, row_new)
            return row_new, rowT

        def load_x(t_):
            xT = xt_pool.tile([128, 128], BF, tag="xT")
            nc.sync.dma_start(out=xT, in_=seq[t_ % T, :, :])
            return xT

        # --- software-pipelined scan ---
        # Prologue: step 0's x-projection + GRU-hidden (on zero state).
        xT = load_x(0)
        pg = psum_pool.tile([128, 512], F32, tag="gi")
        mm_bias(pg, bsb["bias_g"])
        mm_group(pg, xT, wsb["wgi"])
        mm_group_hh(pg, gT, wsb["wgh"])
        g_row, gT = cell(pg, g_row, "g_row")

        for t_ in range(n_steps):
            # AUGRU step t: bias + hidden-side first (depend only on constants
            # and aT(t-1), so the PE pre-pumps them while the GRU cell's
            # elementwise chain runs).
            pa = psum_pool.tile([128, 512], F32, tag="ai")
            mm_bias(pa, bsb["bias_a"])
            mm_group_hh(pa, aT, wsb["wah"])

            if t_ + 1 < n_steps:
                # GRU step t+1 bias + x-projection: ready early, pre-pump too.
                xT = load_x(t_ + 1)
                pg = psum_pool.tile([128, 512], F32, tag="gi")
                mm_bias(pg, bsb["bias_g"])
                mm_group(pg, xT, wsb["wgi"])

            # gT(t)-dependent matmuls: the GRU hidden side first -- it gates
            # the next iteration's critical path.
            if t_ + 1 < n_steps:
                mm_group_hh(pg, gT, wsb["wgh"])
            mm_group(pa, gT, wsb["wai"])
            if t_ + 1 < n_steps:
                # GRU cell t+1 ahead of AUGRU cell t: its output feeds the
                # next iteration's critical path.
                g_row, gT = cell(pg, g_row, "g_row")

            a_row, aT = cell(pa, a_row, "a_row")

        import concourse.bass as bass_mod

        a_f32 = tmp_pool.tile([128, 128], F32, tag="a_f32")
        nc.scalar.activation(a_f32, a_row, mybir.ActivationFunctionType.Copy)
        out_ap = bass_mod.AP(
            tensor=out[:].tensor,
            offset=0,
            ap=[[128, 4], [H, BL], [1, 128]],
        )
        nc.sync.dma_start(out=out_ap, in_=a_f32)

    nc.compile()
    return nc


def _get_program(n_steps=T):
    key = ("prog", n_steps)
    if key not in _CACHE:
        _CACHE[key] = _build_program(n_steps)
    return _CACHE[key]


# ---------------------------------------------------------------------------
# Entry point
# ---------------------------------------------------------------------------

def _make_in_maps(inputs):
    seq_emb = np.asarray(inputs["seq_emb"], np.float32)
    augru_Wih = np.asarray(inputs["augru_Wih"])
    A1 = augru_Wih[:, :H]
    A2 = augru_Wih[:, H:]
    w_fused = (A1 + A2 @ np.asarray(inputs["v_W"])).astype(np.float32)
    b_ai = (np.asarray(inputs["augru_bih"]) + A2 @ np.asarray(inputs["v_b"])).astype(np.float32)
    b_ah = np.asarray(inputs["augru_bhh"], np.float32)
    gru_bih = np.asarray(inputs["gru_bih"])
    gru_bhh = np.asarray(inputs["gru_bhh"])

    consts = {
        "wgi": _arrange_w(np.asarray(inputs["gru_Wih"])),
        "wgh": _arrange_w(np.asarray(inputs["gru_Whh"])),
        "wai": _arrange_w(w_fused),
        "wah": _arrange_w(np.asarray(inputs["augru_Whh"])),
        "bias_g": _bias_row(gru_bih + gru_bhh, gru_bih, gru_bhh),
        "bias_a": _bias_row(b_ai + b_ah, b_ai, b_ah),
    }
    return [
        {"seq": _arrange_seq(seq_emb[c * BL:(c + 1) * BL]), **consts}
        for c in range(N_CORES)
    ]


def _prep_and_run(trace=False, **inputs):
    from concourse.bass_utils import run_bass_kernel_spmd

    in_maps = _make_in_maps(inputs)
    nc = _get_program()
    res = run_bass_kernel_spmd(nc, in_maps, list(range(N_CORES)), trace=trace)
    out = np.concatenate([res.results[c]["out"] for c in range(N_CORES)], axis=0)
    return out.astype(np.float32), res


def kernel(**inputs):
    out, _ = _prep_and_run(**inputs)
    return out


def kernel_traced(**inputs):
    """Like kernel() but profiles the run; returns (output, BassKernelResults)."""
    return _prep_and_run(**inputs, trace=True)


if __name__ == "__main__":
    rng = np.random.default_rng(0)
    ins = {
        "seq_emb": rng.standard_normal((B, T, H), dtype=np.float32),
        "target_emb": rng.standard_normal((B, H), dtype=np.float32),
        "gru_Wih": rng.standard_normal((3 * H, H), dtype=np.float32) * 0.04,
        "gru_Whh": rng.standard_normal((3 * H, H), dtype=np.float32) * 0.04,
        "gru_bih": rng.standard_normal(3 * H).astype(np.float32) * 0.04,
        "gru_bhh": rng.standard_normal(3 * H).astype(np.float32) * 0.04,
        "q_W": rng.standard_normal((H, H), dtype=np.float32) * 0.04,
        "q_b": rng.standard_normal(H).astype(np.float32) * 0.04,
        "k_W": rng.standard_normal((H, H), dtype=np.float32) * 0.04,
        "k_b": rng.standard_normal(H).astype(np.float32) * 0.04,
        "v_W": rng.standard_normal((H, H), dtype=np.float32) * 0.04,
        "v_b": rng.standard_normal(H).astype(np.float32) * 0.04,
        "augru_Wih": rng.standard_normal((3 * H, 2 * H), dtype=np.float32) * 0.04,
        "augru_Whh": rng.standard_normal((3 * H, H), dtype=np.float32) * 0.04,
        "augru_bih": rng.standard_normal(3 * H).astype(np.float32) * 0.04,
        "augru_bhh": rng.standard_normal(3 * H).astype(np.float32) * 0.04,
    }
    o = kernel(**ins)
    print("kernel output", o.shape, o.dtype, float(np.abs(o).max()))
